# revision 7
# baseline (speedup 1.0000x reference)
"""Trainium2 Bass kernel for nn_AudioPreviewModel (topk_masking).

Strategy (8 NeuronCores, SPMD):
  - Batch B=32 sharded 4 per core for features / scores / feat2 / outputs.
  - LSTM + MLP model-parallel over H (each core owns a 128-slice of H and the
    matching 512 rows of the 4096 gate weights).
  - Two small AllGathers per scan step: h2-slice (16 KiB) and feat2 (8 KiB).
  - Key algebraic folds done host-side (exact, linear):
      * weight_norm:  W = g * v / ||v||_row
      * BatchNorm folded into W1/b1
      * Wk2 = W2.T @ Wk  -> scores = scale*(relu_z @ Wk2 + b2@Wk) . f[b,t] + c_b
        (per-batch constant c_b dropped: softmax/argmax invariant)
      * feature_keys never materialized (saves 17 GFLOP + 64 MiB)
  - On-device per step: gates matmuls (cheap stationaries = transposed
    activations), LSTM nonlins, AG(h2^T), z -> relu -> qw chain for own 4
    batch rows, column-tiled concurrent per-b score matvecs, softmax via
    vector.max + Exp(accum), masking via match_replace, col-tiled feat2
    matvecs, AG(feat2^T), indirect-DMA gather of gt rows.
"""

import numpy as np

import concourse.bass as bass
import concourse.bacc as bacc
import concourse.tile as tile
import concourse.mybir as mybir
import concourse.masks as masks
from concourse import bass_utils

F32 = mybir.dt.float32
U32 = mybir.dt.uint32
AF = mybir.ActivationFunctionType
OP = mybir.AluOpType

B, T, D, H, K, C = 32, 1024, 512, 1024, 512, 400
NCORES = 8
BPC = B // NCORES          # 4 batch rows per core
CS = C // NCORES           # 50 step_pred cols per core
SCALE = float(1.0 / np.sqrt(np.float32(K)))
NEG = -1.0e35

_CACHE = {}


def _build(E):
    nc = bacc.Bacc(None, target_bir_lowering=False)

    dt_in = {}
    def din(name, shape, dt=F32):
        dt_in[name] = nc.dram_tensor(name, shape, dt, kind="ExternalInput")
        return dt_in[name]

    fN_d = din("fN", (BPC, T, D))                 # features own, natural
    fT_d = din("fT", (BPC, D, T))                 # features own, transposed
    gt_d = din("gt", (BPC * T, C))                # gt rows own, flattened
    wih_d = din("wih", (D, 512))                  # W_ih.T own gate rows
    whh_d = din("whh", (H, 512))                  # W_hh.T own gate rows
    w1_d = din("w1", (H, H))                      # W1'(bn-folded).T, full
    wk2_d = din("wk2", (H, K))                    # W2.T@Wk, full
    wpt_d = din("wpt", (H, CS))                   # Wp.T own cols
    wct_d = din("wct", (D, C))                    # Wc.T full
    bg_d = din("bg", (1, 512))                    # b_ih+b_hh own rows
    b1f_d = din("b1f", (1, H))                    # bn-folded b1
    b2k_d = din("b2k", (1, K))                    # b2@Wk
    bp_d = din("bp", (1, CS))
    bc_d = din("bc", (BPC, C))                    # bc replicated rows
    f0T_d = din("f0T", (D, B))                    # feat0.T (host mean feature)
    ma0_d = din("ma0", (BPC, T))                  # initial additive mask
    rb_d = din("rb", (BPC, 1), U32)               # gather row base b*T
    sel_d = din("sel", (B, BPC))                  # one-hot own-batch selector

    sp_o = nc.dram_tensor("sp_o", (E, B, CS), F32, kind="ExternalOutput")
    pred_o = nc.dram_tensor("pred_o", (BPC, C), F32, kind="ExternalOutput")
    gtp_o = nc.dram_tensor("gtp_o", (BPC, C), F32, kind="ExternalOutput")

    RG = [list(range(NCORES))]

    with tile.TileContext(nc) as tc:
        with (
            tc.tile_pool(name="const", bufs=1) as cp,
            tc.tile_pool(name="state", bufs=1) as st,
            tc.tile_pool(name="work", bufs=1) as wk,
            tc.tile_pool(name="stream", bufs=2) as sm,
            tc.tile_pool(name="psum", bufs=1, space="PSUM") as ps,
            tc.tile_pool(name="dram", bufs=2, space="DRAM") as dr,
        ):
            ident = cp.tile([128, 128], F32)
            masks.make_identity(nc, ident[:])
            ones = cp.tile([1, 32], F32)
            nc.vector.memset(ones[:], 1.0)

            # ---- resident loads (small first, then features) ----
            wih = cp.tile([128, 4 * 512], F32)
            for kt in range(4):
                nc.sync.dma_start(wih[:, kt * 512:(kt + 1) * 512],
                                  wih_d[kt * 128:(kt + 1) * 128, :])
            whh = cp.tile([128, 8 * 512], F32)
            for kt in range(8):
                nc.sync.dma_start(whh[:, kt * 512:(kt + 1) * 512],
                                  whh_d[kt * 128:(kt + 1) * 128, :])
            wpt = cp.tile([128, 8 * CS], F32)
            for kt in range(8):
                nc.sync.dma_start(wpt[:, kt * CS:(kt + 1) * CS],
                                  wpt_d[kt * 128:(kt + 1) * 128, :])
            bg = cp.tile([1, 512], F32); nc.sync.dma_start(bg[:], bg_d[:])
            b1f = cp.tile([1, H], F32); nc.sync.dma_start(b1f[:], b1f_d[:])
            b2k = cp.tile([1, K], F32); nc.sync.dma_start(b2k[:], b2k_d[:])
            bp = cp.tile([1, CS], F32); nc.sync.dma_start(bp[:], bp_d[:])
            bcr = cp.tile([BPC, C], F32); nc.sync.dma_start(bcr[:], bc_d[:])
            rb = cp.tile([BPC, 1], U32); nc.sync.dma_start(rb[:], rb_d[:])
            sel = cp.tile([B, BPC], F32); nc.sync.dma_start(sel[:], sel_d[:])

            fTs = cp.tile([128, BPC * 4 * T], F32)   # [p, b*4096 + kt*1024 + t]
            for b in range(BPC):
                for kt in range(4):
                    nc.sync.dma_start(
                        fTs[:, (b * 4 + kt) * T:(b * 4 + kt + 1) * T],
                        fT_d[b, kt * 128:(kt + 1) * 128, :])
            fNs = cp.tile([128, BPC * 8 * D], F32)   # [p, (b*8+tt)*512 + d]
            for b in range(BPC):
                for tt in range(8):
                    nc.sync.dma_start(
                        fNs[:, (b * 8 + tt) * D:(b * 8 + tt + 1) * D],
                        fN_d[b, tt * 128:(tt + 1) * 128, :])

            # ---- state ----
            hT = st.tile([128, 8 * 32], F32)      # h.T full [H, B]
            featT = st.tile([128, 4 * 32], F32)   # feat.T full [D, B]
            c_sb = st.tile([32, 128], F32)        # own H-slice cell state
            M_add = st.tile([BPC, T], F32)
            gts = st.tile([BPC, C], F32)          # sum of gathered gt rows
            pfT = st.tile([128, 4 * BPC], F32)    # sum of feat2.T
            nc.vector.memset(hT[:], 0.0)
            nc.vector.memset(c_sb[:], 0.0)
            nc.vector.memset(gts[:], 0.0)
            nc.vector.memset(pfT[:], 0.0)
            nc.sync.dma_start(M_add[:], ma0_d[:])
            nc.sync.dma_start(
                featT[:].rearrange("p (dt j) -> p dt j", dt=4),
                f0T_d[:].rearrange("(dt p) j -> p dt j", p=128))

            for e in range(E):
                # ---------- gates ----------
                G = ps.tile([32, 512], F32, tag="g")
                nc.tensor.matmul(G[:], ones[:1, :32], bg[:1, :],
                                 start=True, stop=False)
                for kt in range(4):
                    nc.tensor.matmul(G[:], featT[:, kt * 32:(kt + 1) * 32],
                                     wih[:, kt * 512:(kt + 1) * 512],
                                     start=False, stop=False)
                for kt in range(8):
                    nc.tensor.matmul(G[:], hT[:, kt * 32:(kt + 1) * 32],
                                     whh[:, kt * 512:(kt + 1) * 512],
                                     start=False, stop=(kt == 7))
                # ---------- LSTM nonlinearities ----------
                sigi = wk.tile([32, 128], F32, tag="sigi")
                sigf = wk.tile([32, 128], F32, tag="sigf")
                tang = wk.tile([32, 128], F32, tag="tang")
                sigo = wk.tile([32, 128], F32, tag="sigo")
                nc.scalar.activation(sigi[:], G[:, 0:128], AF.Sigmoid)
                nc.scalar.activation(sigf[:], G[:, 128:256], AF.Sigmoid)
                nc.scalar.activation(tang[:], G[:, 256:384], AF.Tanh)
                nc.scalar.activation(sigo[:], G[:, 384:512], AF.Sigmoid)
                t1 = wk.tile([32, 128], F32, tag="t1")
                nc.vector.tensor_mul(t1[:], sigi[:], tang[:])
                t2 = wk.tile([32, 128], F32, tag="t2")
                nc.vector.tensor_mul(t2[:], sigf[:], c_sb[:])
                nc.vector.tensor_add(c_sb[:], t1[:], t2[:])
                tanc = wk.tile([32, 128], F32, tag="tanc")
                nc.scalar.activation(tanc[:], c_sb[:], AF.Tanh)
                h2 = wk.tile([32, 128], F32, tag="h2")
                nc.vector.tensor_mul(h2[:], sigo[:], tanc[:])
                # ---------- h2 -> h2T, AllGather ----------
                TRh = ps.tile([128, 32], F32, tag="tr")
                nc.tensor.transpose(TRh[:], h2[:], ident[:32, :32])
                h2t = wk.tile([128, 32], F32, tag="h2t")
                nc.vector.tensor_copy(h2t[:], TRh[:])
                ag1i = dr.tile([128, 32], F32)
                nc.sync.dma_start(ag1i[:], h2t[:])
                ag1o = dr.tile([1024, 32], F32)
                nc.gpsimd.collective_compute(
                    "AllGather", OP.bypass, ins=[ag1i.opt()], outs=[ag1o.opt()],
                    replica_groups=RG)
                nc.sync.dma_start(
                    hT[:].rearrange("p (k j) -> p k j", k=8),
                    ag1o[:].rearrange("(k p) j -> p k j", p=128))
                # ---------- step_pred (off critical path) ----------
                SP = ps.tile([32, 512], F32, tag="g")
                nc.tensor.matmul(SP[:, :CS], ones[:1, :32], bp[:1, :],
                                 start=True, stop=False)
                for kt in range(8):
                    nc.tensor.matmul(SP[:, :CS], hT[:, kt * 32:(kt + 1) * 32],
                                     wpt[:, kt * CS:(kt + 1) * CS],
                                     start=False, stop=(kt == 7))
                spb = wk.tile([32, CS], F32, tag="spb")
                nc.vector.tensor_copy(spb[:], SP[:, :CS])
                nc.sync.dma_start(sp_o[e], spb[:])
                # ---------- z = h2 @ W1'.T (+b1'), relu (all 32 rows) ----------
                Z = ps.tile([32, 1024], F32, tag="z")
                for cc in range(2):
                    zr = Z[:, cc * 512:(cc + 1) * 512]
                    nc.tensor.matmul(zr, ones[:1, :32],
                                     b1f[:1, cc * 512:(cc + 1) * 512],
                                     start=True, stop=False)
                    for kt in range(8):
                        w1c = sm.tile([128, 512], F32, tag="w1c")
                        nc.sync.dma_start(
                            w1c[:], w1_d[kt * 128:(kt + 1) * 128,
                                         cc * 512:(cc + 1) * 512])
                        nc.tensor.matmul(zr, hT[:, kt * 32:(kt + 1) * 32],
                                         w1c[:], start=False, stop=(kt == 7))
                rz = wk.tile([32, 1024], F32, tag="rz")
                nc.vector.tensor_scalar_max(rz[:], Z[:], 0.0)
                # ---------- rz -> rzT (all rows) ----------
                TRz = ps.tile([128, 8 * 32], F32, tag="tr")
                for kt in range(8):
                    nc.tensor.transpose(TRz[:, kt * 32:(kt + 1) * 32],
                                        rz[:, kt * 128:(kt + 1) * 128],
                                        ident[:32, :32])
                rzT = wk.tile([128, 8 * 32], F32, tag="rzT")
                nc.vector.tensor_copy(rzT[:], TRz[:])
                # ---------- qw = rz @ Wk2 (+b2k), scaled, then select own ----------
                QW = ps.tile([32, 512], F32, tag="qw")
                nc.tensor.matmul(QW[:], ones[:1, :32], b2k[:1, :],
                                 start=True, stop=False)
                for kt in range(8):
                    wkc = sm.tile([128, 512], F32, tag="wkc")
                    nc.sync.dma_start(wkc[:],
                                      wk2_d[kt * 128:(kt + 1) * 128, :])
                    nc.tensor.matmul(QW[:], rzT[:, kt * 32:(kt + 1) * 32],
                                     wkc[:], start=False, stop=(kt == 7))
                qwa = wk.tile([32, 512], F32, tag="scf")
                nc.vector.tensor_scalar_mul(qwa[:], QW[:], SCALE)
                QO = ps.tile([4, 512], F32, tag="f2")
                nc.tensor.matmul(QO[:], sel[:], qwa[:], start=True, stop=True)
                qw = wk.tile([4, 512], F32, tag="qw_sb")
                nc.vector.tensor_copy(qw[:], QO[:])
                TRq = ps.tile([128, 16], F32, tag="tr")
                for dk in range(4):
                    nc.tensor.transpose(TRq[:, dk * 4:(dk + 1) * 4],
                                        qw[:, dk * 128:(dk + 1) * 128],
                                        ident[:4, :4])
                qwT = wk.tile([128, 16], F32, tag="qwT")
                nc.vector.tensor_copy(qwT[:], TRq[:])
                # ---------- scores (col-tiled, 4 concurrent) ----------
                SC = ps.tile([128, 1024], F32, tag="sc")
                for cc in range(2):
                    for kt in range(4):
                        for b in range(BPC):
                            nc.tensor.matmul(
                                SC[32 * b:32 * b + 1,
                                   cc * 512:(cc + 1) * 512],
                                qwT[:, kt * 4 + b:kt * 4 + b + 1],
                                fTs[:, (b * 4 + kt) * T + cc * 512:
                                       (b * 4 + kt) * T + (cc + 1) * 512],
                                start=(kt == 0), stop=(kt == 3),
                                tile_position=(0, 32 * b))
                scf = wk.tile([128, 1024], F32, tag="scf")
                nc.vector.tensor_copy(scf[:], SC[:])
                ssb = wk.tile([4, 1024], F32, tag="ssb")
                nc.sync.dma_start(ssb[:], scf[::32, :])
                masked = wk.tile([4, 1024], F32, tag="masked")
                nc.vector.tensor_add(masked[:], ssb[:], M_add[:])
                # ---------- softmax pieces ----------
                mx8 = wk.tile([4, 8], F32, tag="mx8")
                nc.vector.max(out=mx8[:], in_=masked[:])
                ngm = wk.tile([4, 1], F32, tag="ngm")
                nc.vector.tensor_scalar_mul(ngm[:], mx8[:, 0:1], -1.0)
                ex = wk.tile([4, 1024], F32, tag="rz")
                sums = wk.tile([4, 1], F32, tag="sums")
                nc.scalar.activation(ex[:], masked[:], AF.Exp,
                                     bias=ngm[:], scale=1.0, accum_out=sums[:])
                rcp = wk.tile([4, 1], F32, tag="rcp")
                nc.vector.reciprocal(rcp[:], sums[:])
                idx8 = wk.tile([4, 8], U32, tag="idx8")
                nc.vector.max_index(idx8[:], mx8[:], masked[:])
                # ---------- mask update (next step) ----------
                mri = wk.tile([4, 8], F32, tag="mri")
                nc.vector.memset(mri[:], 1.0e38)
                nc.vector.tensor_copy(mri[:, 0:1], mx8[:, 0:1])
                repl = wk.tile([4, 1024], F32, tag="ssb")
                nc.vector.match_replace(out=repl[:], in_to_replace=mri[:],
                                        in_values=masked[:], imm_value=NEG)
                nc.vector.tensor_scalar(M_add[:], repl[:], -1.0e34, NEG,
                                        OP.is_lt, OP.mult)
                # ---------- gt gather (off critical path) ----------
                gidx = wk.tile([4, 1], U32, tag="gidx")
                nc.vector.tensor_tensor(out=gidx[:], in0=idx8[:, 0:1],
                                        in1=rb[:], op=OP.add)
                gtg = wk.tile([4, C], F32, tag="gtg")
                nc.gpsimd.indirect_dma_start(
                    out=gtg[:], out_offset=None, in_=gt_d[:],
                    in_offset=bass.IndirectOffsetOnAxis(ap=gidx[:, :1], axis=0))
                nc.vector.tensor_add(gts[:], gts[:], gtg[:])
                # ---------- ex -> exT, feat2 ----------
                TRe = ps.tile([128, 32], F32, tag="tr")
                for tt in range(8):
                    nc.tensor.transpose(TRe[:, tt * 4:(tt + 1) * 4],
                                        ex[:, tt * 128:(tt + 1) * 128],
                                        ident[:4, :4])
                exT = wk.tile([128, 32], F32, tag="exT")
                nc.vector.tensor_copy(exT[:], TRe[:])
                F2 = ps.tile([128, 512], F32, tag="f2")
                for tt in range(8):
                    for b in range(BPC):
                        nc.tensor.matmul(
                            F2[32 * b:32 * b + 1, :],
                            exT[:, tt * 4 + b:tt * 4 + b + 1],
                            fNs[:, (b * 8 + tt) * D:(b * 8 + tt + 1) * D],
                            start=(tt == 0), stop=(tt == 7),
                            tile_position=(0, 32 * b))
                f2f = wk.tile([128, 512], F32, tag="scf")
                nc.vector.tensor_copy(f2f[:], F2[:])
                f2 = wk.tile([4, 512], F32, tag="f2sb")
                nc.sync.dma_start(f2[:], f2f[::32, :])
                f2n = f2
                nc.vector.tensor_scalar_mul(f2n[:], f2[:], rcp[:])
                TRf = ps.tile([128, 16], F32, tag="tr")
                for dk in range(4):
                    nc.tensor.transpose(TRf[:, dk * 4:(dk + 1) * 4],
                                        f2n[:, dk * 128:(dk + 1) * 128],
                                        ident[:4, :4])
                f2t = wk.tile([128, 16], F32, tag="f2t")
                nc.vector.tensor_copy(f2t[:], TRf[:])
                nc.vector.tensor_add(pfT[:], pfT[:], f2t[:])
                # ---------- AG2: feat2 -> featT ----------
                ag2i = dr.tile([512, BPC], F32)
                nc.sync.dma_start(
                    ag2i[:].rearrange("(dk p) j -> p dk j", p=128),
                    f2t[:].rearrange("p (dk j) -> p dk j", dk=4))
                ag2o = dr.tile([512 * NCORES, BPC], F32)
                nc.gpsimd.collective_compute(
                    "AllGather", OP.bypass, ins=[ag2i.opt()], outs=[ag2o.opt()],
                    replica_groups=RG)
                fv = featT[:].rearrange("p (dt m j) -> p dt m j", dt=4, m=8)
                for m in range(8):
                    nc.sync.dma_start(
                        fv[:, :, m, :],
                        ag2o[m * 512:(m + 1) * 512, :]
                            .rearrange("(dt p) j -> p dt j", p=128))

            # ---------- end: predictions ----------
            wctA = wk.tile([128, 2 * C], F32, tag="ssb")
            wctB = wk.tile([128, 2 * C], F32, tag="masked")
            for dk in range(4):
                w = wctA if dk < 2 else wctB
                nc.sync.dma_start(w[:, (dk % 2) * C:(dk % 2 + 1) * C],
                                  wct_d[dk * 128:(dk + 1) * 128, :])
            PR = ps.tile([4, 512], F32, tag="qw")
            for dk in range(4):
                w = wctA if dk < 2 else wctB
                nc.tensor.matmul(PR[:, :C], pfT[:, dk * 4:(dk + 1) * 4],
                                 w[:, (dk % 2) * C:(dk % 2 + 1) * C],
                                 start=(dk == 0), stop=(dk == 3))
            prs = wk.tile([4, C], F32, tag="gtg")
            nc.vector.scalar_tensor_tensor(
                out=prs[:], in0=PR[:, :C], scalar=1.0 / E, in1=bcr[:],
                op0=OP.mult, op1=OP.add)
            nc.sync.dma_start(pred_o[:], prs[:])
            gpm = wk.tile([4, C], F32, tag="gtg")
            nc.vector.tensor_scalar_mul(gpm[:], gts[:], 1.0 / E)
            nc.sync.dma_start(gtp_o[:], gpm[:])

    nc.finalize()
    return nc


def _prep(inputs):
    """Host-side exact algebraic folds + per-core sharding."""
    g = lambda n: np.asarray(inputs[n], dtype=np.float32)
    features = g("features")
    gt = g("input_gt_predictions")
    fm = g("feature_masks")
    sr = g("start_rand")
    v_ih, g_ih, b_ih = g("v_ih"), g("g_ih"), g("b_ih")
    v_hh, g_hh, b_hh = g("v_hh"), g("g_hh"), g("b_hh")
    W1, b1 = g("W1"), g("b1")
    bn_g, bn_b = g("bn_gamma"), g("bn_beta")
    bn_m, bn_v = g("bn_mean"), g("bn_var")
    W2, b2 = g("W2"), g("b2")
    Wp, bp = g("Wp"), g("bp")
    Wk, bk = g("Wk"), g("bk")
    Wc, bc = g("Wc"), g("bc")
    E = int(np.asarray(inputs["episode_length"]))

    W_ih = g_ih * v_ih / np.linalg.norm(v_ih, axis=1, keepdims=True)
    W_hh = g_hh * v_hh / np.linalg.norm(v_hh, axis=1, keepdims=True)
    bg = b_ih + b_hh
    A = bn_g / np.sqrt(bn_v + 1e-5)
    W1f = (W1 * A[:, None]).astype(np.float32)
    b1f = ((b1 - bn_m) * A + bn_b).astype(np.float32)
    Wk2 = (W2.T @ Wk).astype(np.float32)
    b2k = (b2 @ Wk).astype(np.float32)

    feat0 = features.sum(axis=1) / fm.sum(axis=1, keepdims=True)
    idx0 = np.argmax(sr + fm, axis=1)
    M0 = ((1.0 - fm) * NEG).astype(np.float32)
    M0[np.arange(B), idx0] = NEG

    w1t = np.ascontiguousarray(W1f.T)
    wk2 = np.ascontiguousarray(Wk2)
    wct = np.ascontiguousarray(Wc.T)
    f0T = np.ascontiguousarray(feat0.T)

    per_core = []
    for m in range(NCORES):
        bs = slice(m * BPC, (m + 1) * BPC)
        gsel = np.concatenate(
            [np.arange(gg * H + m * 128, gg * H + (m + 1) * 128)
             for gg in range(4)])
        cs = slice(m * CS, (m + 1) * CS)
        per_core.append({
            "fN": np.ascontiguousarray(features[bs]),
            "fT": np.ascontiguousarray(features[bs].transpose(0, 2, 1)),
            "gt": np.ascontiguousarray(gt[bs].reshape(BPC * T, C)),
            "wih": np.ascontiguousarray(W_ih[gsel].T),
            "whh": np.ascontiguousarray(W_hh[gsel].T),
            "w1": w1t,
            "wk2": wk2,
            "wpt": np.ascontiguousarray(Wp[cs].T),
            "wct": wct,
            "bg": np.ascontiguousarray(bg[gsel][None, :]),
            "b1f": b1f[None, :],
            "b2k": b2k[None, :],
            "bp": np.ascontiguousarray(bp[cs][None, :]),
            "bc": np.tile(bc[None, :], (BPC, 1)),
            "f0T": f0T,
            "ma0": np.ascontiguousarray(M0[bs]),
            "rb": (np.arange(BPC, dtype=np.uint32) * T)[:, None],
            "sel": np.eye(B, dtype=np.float32)[:, m * BPC:(m + 1) * BPC].copy(),
        })
    return E, per_core


def kernel(**inputs):
    E, per_core = _prep(inputs)
    if E not in _CACHE:
        _CACHE[E] = _build(E)
    nc = _CACHE[E]
    res = bass_utils.run_bass_kernel_spmd(
        nc, per_core, core_ids=list(range(NCORES)))
    outs = res.results
    predictions = np.concatenate([outs[m]["pred_o"] for m in range(NCORES)], 0)
    gt_predictions = np.concatenate([outs[m]["gtp_o"] for m in range(NCORES)], 0)
    step_preds = np.concatenate([outs[m]["sp_o"] for m in range(NCORES)], 2)
    return predictions, gt_predictions, step_preds
